# revision 1
# baseline (speedup 1.0000x reference)
"""Trainium2 Bass kernel for a 3-layer GraphConv GNN (nn_CustomGNN_34050500722941).

Reference computation (per layer, PyG GraphConv aggr='add'):
    h = relu(x @ preW.T + preb)
    3x: h = relu(segment_sum(h[src], dst) @ relW.T + relb + h @ rootW.T)
    out = relu(h @ postW.T + postb)

Strategy (8 NeuronCores, SPMD):
  - Nodes are sharded by range: core c owns nodes [c*NL, (c+1)*NL).
  - The full node-feature table h is replicated in each core's DRAM via
    AllGather once per layer (node-major rows of 64 fp32 = 256B).
  - Each core processes the edges whose dst lands in its range:
      * messages h[src] are fetched with GPSIMD dma_gather (one 256B row
        per edge).  Gather indices are int16, so the table is split into
        NCHUNK row-ranges (<=32767 rows each) and edges are grouped by the
        chunk their src falls into.
      * the scatter-add over dst is performed on the PE: edges are sorted
        by 128-wide dst block; for each 128-edge tile a one-hot matrix
        S[e, dst_rel] (built on DVE with is_equal against an iota) maps
        messages into a PSUM accumulator [64 feat, 128 dst] (feature-major).
      * dense part: h_new.T = relu(relW.T.T @ agg.T + rootW.T.T @ h.T + b)
        entirely feature-major on the PE, no transposes needed.
  - Host does index preprocessing (edge bucketing/padding) and the final
    unshard/transpose.
"""

import math
import numpy as np


# ----------------------------------------------------------------------------
# Host-side preprocessing
# ----------------------------------------------------------------------------

def _plan(n_nodes, n_edges, n_cores):
    NL = ((n_nodes + n_cores - 1) // n_cores + 511) // 512 * 512
    NPAD = NL * n_cores
    NCHUNK = 1
    while NPAD // NCHUNK > 32512 or NPAD % NCHUNK:
        NCHUNK += 1
    CH = NPAD // NCHUNK
    NBLK = NL // 128
    NDCH = NL // 512
    return dict(NL=NL, NPAD=NPAD, NCHUNK=NCHUNK, CH=CH, NBLK=NBLK, NDCH=NDCH)


def _preprocess_edges(edge_index, n_cores, plan):
    """Bucket edges by (core, chunk, block); build gather index / rel-dst
    arrays with a layout shared by all cores (cell sizes = max over cores)."""
    NL, NCHUNK, CH, NBLK = plan["NL"], plan["NCHUNK"], plan["CH"], plan["NBLK"]
    SE = 1024  # edges per gather slice (SWDGE ring holds 1024 descriptors)

    src = np.asarray(edge_index[0], dtype=np.int64)
    dst = np.asarray(edge_index[1], dtype=np.int64)

    core = dst // NL
    chunk = src // CH
    src_local = (src % CH).astype(np.int32)
    dst_local = dst % NL
    block = (dst_local // 128).astype(np.int32)
    rel = (dst_local % 128).astype(np.int32)

    # counts[core, chunk, block]
    key = (core * NCHUNK + chunk) * NBLK + block
    counts = np.bincount(key, minlength=n_cores * NCHUNK * NBLK).reshape(
        n_cores, NCHUNK, NBLK
    )
    # tiles per cell, shared across cores
    T_bk = (counts.max(axis=0) + 127) // 128  # [NCHUNK, NBLK]
    cell_slots = T_bk * 128
    # slot offset of cell (k, b) within chunk-stream k
    off_bk = np.zeros((NCHUNK, NBLK), dtype=np.int64)
    off_bk[:, 1:] = np.cumsum(cell_slots, axis=1)[:, :-1]
    L_used = cell_slots.sum(axis=1)  # [NCHUNK] used slots per stream
    L_k = ((L_used + SE - 1) // SE * SE).astype(np.int64)  # padded to slices
    L_k = np.maximum(L_k, SE)

    # rank of each edge within its (core, chunk, block) cell
    order = np.lexsort((block, chunk, core))
    key_sorted = key[order]
    grp_start = np.zeros(len(key_sorted), dtype=np.int64)
    new_grp = np.empty(len(key_sorted), dtype=bool)
    if len(key_sorted):
        new_grp[0] = True
        new_grp[1:] = key_sorted[1:] != key_sorted[:-1]
        starts = np.flatnonzero(new_grp)
        grp_start = starts[np.cumsum(new_grp) - 1]
    rank_sorted = np.arange(len(key_sorted)) - grp_start

    rank = np.empty(len(src), dtype=np.int64)
    rank[order] = rank_sorted

    slot = off_bk[chunk, block] + rank  # slot within chunk-stream

    # Per-core arrays
    eidx = []  # [n_cores][NCHUNK] int16 [128, L_k/16]
    relv = []  # [n_cores] bf16-able fp32 [128, Ttot]
    TOFF = np.zeros(NCHUNK + 1, dtype=np.int64)
    for k in range(NCHUNK):
        TOFF[k + 1] = TOFF[k] + L_k[k] // 128
    Ttot = int(TOFF[NCHUNK])

    for c in range(n_cores):
        m_c = core == c
        e_c = []
        rv = np.full(Ttot * 128, -1.0, dtype=np.float32)
        for k in range(NCHUNK):
            m = m_c & (chunk == k)
            arr = np.zeros(L_k[k], dtype=np.int16)  # cell pads -> row 0
            arr[int(L_used[k]):] = -1  # stream tail -> skipped
            arr[slot[m]] = src_local[m].astype(np.int16)
            wrapped = arr.reshape(-1, 16).T  # [16, L/16]
            e_c.append(np.tile(wrapped, (8, 1)).copy())  # [128, L/16]
            rv[TOFF[k] * 128 + slot[m]] = rel[m]
        eidx.append(e_c)
        # slot i -> (partition i%128, tile i//128)
        import ml_dtypes

        relv.append(
            np.ascontiguousarray(rv.reshape(Ttot, 128).T).astype(ml_dtypes.bfloat16)
        )

    return dict(
        SE=SE,
        eidx=eidx,
        relv=relv,
        T_bk=T_bk.astype(int),
        off_bk=off_bk,
        L_used=L_used.astype(int),
        L_k=L_k.astype(int),
        TOFF=TOFF,
        Ttot=Ttot,
    )


# ----------------------------------------------------------------------------
# Device program
# ----------------------------------------------------------------------------

def _build_program(plan, ep, n_cores, n_mp, in_dim, out_dim, hidden, skip=(), repeats=1):
    import concourse.bass as bass
    import concourse.bacc as bacc
    import concourse.mybir as mybir
    from concourse import tile

    f32 = mybir.dt.float32
    bf16 = mybir.dt.bfloat16
    i16 = mybir.dt.int16
    H = hidden
    NL, NPAD, NCHUNK, CH = plan["NL"], plan["NPAD"], plan["NCHUNK"], plan["CH"]
    NBLK, NDCH = plan["NBLK"], plan["NDCH"]
    SE = ep["SE"]
    T_bk, off_bk, L_k, L_used = ep["T_bk"], ep["off_bk"], ep["L_k"], ep["L_used"]
    TOFF, Ttot = ep["TOFF"], ep["Ttot"]
    SB = 8  # tiles per S-build op (one gather slice)
    TPS = SE // 128  # tiles per slice (20)

    nc = bacc.Bacc(None, target_bir_lowering=False, num_devices=n_cores)
    rg = [list(range(n_cores))]

    # ---- I/O ----
    xT_d = nc.dram_tensor("xT", [in_dim, NL], f32, kind="ExternalInput")
    eidx_d = [
        nc.dram_tensor(f"eidx{k}", [128, int(L_k[k]) // 16], i16, kind="ExternalInput")
        for k in range(NCHUNK)
    ]
    relv_d = nc.dram_tensor("relv", [128, Ttot], bf16, kind="ExternalInput")
    iota_d = nc.dram_tensor("iota", [128, 128], bf16, kind="ExternalInput")
    ident_d = nc.dram_tensor("ident", [H, H], f32, kind="ExternalInput")
    preWT_d = nc.dram_tensor("preWT", [in_dim, H], f32, kind="ExternalInput")
    preb_d = nc.dram_tensor("preb", [H, 1], f32, kind="ExternalInput")
    relWT_d = [
        nc.dram_tensor(f"relWT{l}", [H, H], f32, kind="ExternalInput")
        for l in range(n_mp)
    ]
    rootWT_d = [
        nc.dram_tensor(f"rootWT{l}", [H, H], f32, kind="ExternalInput")
        for l in range(n_mp)
    ]
    relb_d = [
        nc.dram_tensor(f"relb{l}", [H, 1], f32, kind="ExternalInput")
        for l in range(n_mp)
    ]
    postWT_d = nc.dram_tensor("postWT", [H, out_dim], f32, kind="ExternalInput")
    postb_d = nc.dram_tensor("postb", [out_dim, 1], f32, kind="ExternalInput")
    outT_d = nc.dram_tensor("outT", [out_dim, NL], f32, kind="ExternalOutput")

    # ---- internal DRAM (h tables per layer) ----
    tbl_loc = [nc.dram_tensor(f"tbl_loc{l}", [NL, H], f32) for l in range(n_mp)]
    tbl = [
        nc.dram_tensor(f"tbl{l}", [NPAD, H], f32, addr_space="Shared")
        for l in range(n_mp)
    ]

    with tile.TileContext(nc) as tc:
        with (
            tc.tile_pool(name="const", bufs=1) as constp,
            tc.tile_pool(name="big", bufs=1) as bigp,
            tc.tile_pool(name="msg", bufs=2) as msgp,
            tc.tile_pool(name="msgb", bufs=6) as msgbp,
            tc.tile_pool(name="sbuild", bufs=6) as sp,
            tc.tile_pool(name="eix", bufs=4) as eixp,
            tc.tile_pool(name="small", bufs=3) as smallp,
            tc.tile_pool(name="io", bufs=2) as iop,
            tc.tile_pool(name="aggps", bufs=4, space="PSUM") as aggps,
            tc.tile_pool(name="dps", bufs=2, space="PSUM") as dps,
            tc.tile_pool(name="tps", bufs=2, space="PSUM") as tps,
        ):
            # ---- resident constants ----
            iota_t = constp.tile([128, 128], bf16, tag="iota")
            nc.sync.dma_start(out=iota_t[:], in_=iota_d[:])
            ident_t = constp.tile([H, H], f32, tag="ident")
            nc.sync.dma_start(out=ident_t[:], in_=ident_d[:])
            preWT_t = constp.tile([in_dim, H], f32, tag="preWT")
            nc.sync.dma_start(out=preWT_t[:], in_=preWT_d[:])
            preb_t = constp.tile([H, 1], f32, tag="preb")
            nc.sync.dma_start(out=preb_t[:], in_=preb_d[:])
            postWT_t = constp.tile([H, out_dim], f32, tag="postWT")
            nc.sync.dma_start(out=postWT_t[:], in_=postWT_d[:])
            postb_t = constp.tile([out_dim, 1], f32, tag="postb")
            nc.sync.dma_start(out=postb_t[:], in_=postb_d[:])
            relWT_t, rootWT_t, relb_t = [], [], []
            for l in range(n_mp):
                w1 = constp.tile([H, H], f32, tag=f"relWT{l}")
                nc.sync.dma_start(out=w1[:], in_=relWT_d[l][:])
                w2 = constp.tile([H, H], f32, tag=f"rootWT{l}")
                nc.sync.dma_start(out=w2[:], in_=rootWT_d[l][:])
                b1 = constp.tile([H, 1], f32, tag=f"relb{l}")
                nc.sync.dma_start(out=b1[:], in_=relb_d[l][:])
                relWT_t.append(w1)
                rootWT_t.append(w2)
                relb_t.append(b1)
            relv_t = constp.tile([128, Ttot], bf16, tag="relv")
            nc.sync.dma_start(out=relv_t[:], in_=relv_d[:])

            hT_t = bigp.tile([H, NL], f32, tag="hT")
            aggT_t = bigp.tile([H, NL], f32, tag="aggT")
            hT = hT_t[:, :]
            aggT = aggT_t[:, :]

            # ---- pre-MP dense ----
            for i in range(NDCH):
                xt = iop.tile([in_dim, 512], f32, tag="xt")
                nc.sync.dma_start(out=xt[:], in_=xT_d[:, i * 512 : (i + 1) * 512])
                ps = dps.tile([64, 512], f32, tag="dps")
                nc.tensor.matmul(ps[0:H, :], preWT_t[:], xt[:], start=True, stop=True)
                nc.scalar.activation(
                    hT[:, i * 512 : (i + 1) * 512],
                    ps[0:H, :],
                    mybir.ActivationFunctionType.Relu,
                    bias=preb_t[:],
                )

            # ---- message-passing layers ----
            for l in [l_ for _ in range(repeats) for l_ in range(n_mp)]:
                # 1) node-major h table rows: transpose hT block-wise, DMA out
                tblr = tbl_loc[l].rearrange("(j p) f -> p j f", p=128)  # [128,NBLK,H]
                for j0 in range(0, NBLK, 4):
                    jn = min(4, NBLK - j0)
                    st = smallp.tile([128, 4, H], f32, tag="tblw")
                    for j in range(j0, j0 + jn):
                        pt = tps.tile([128, H], f32, tag="tps")
                        nc.tensor.transpose(
                            pt[:], hT[:, j * 128 : (j + 1) * 128], ident_t[:]
                        )
                        nc.scalar.activation(
                            st[:, j - j0, :],
                            pt[:],
                            mybir.ActivationFunctionType.Copy,
                        )
                    nc.sync.dma_start(
                        out=tblr[:, j0 : j0 + jn, :], in_=st[:, 0:jn, :]
                    )
                # 2) replicate across cores
                if "ag" not in skip:
                    nc.gpsimd.collective_compute(
                    "AllGather",
                    mybir.AluOpType.bypass,
                        replica_groups=rg,
                        ins=[tbl_loc[l][:]],
                        outs=[tbl[l][:]],
                    )
                # 3) gather + S-build + segment-matmul, block-major schedule
                msgb_tiles = [dict() for _ in range(NCHUNK)]  # slice -> bf16 tile
                s_tiles = [dict() for _ in range(NCHUNK)]  # sb-idx -> S tile

                def ensure_slice(k, sl):
                    if sl in msgb_tiles[k]:
                        return
                    n_sl = int(L_used[k] + SE - 1) // SE
                    if sl >= n_sl:
                        return
                    rem = int(L_used[k]) - sl * SE
                    cnt = min(SE, rem)
                    et = eixp.tile([128, SE // 16], i16, tag="eix")
                    nc.sync.dma_start(
                        out=et[:],
                        in_=eidx_d[k][:, sl * (SE // 16) : (sl + 1) * (SE // 16)],
                    )
                    mt = msgp.tile([128, TPS, H], f32, tag="msg")
                    if cnt < SE:
                        nc.vector.memset(mt[:], 0.0)
                    if "gather" in skip:
                        msgb_tiles[k][sl] = None
                    else:
                        nc.gpsimd.dma_gather(
                        out_ap=mt[:].bitcast(bf16),
                        in_ap=tbl[l][k * CH : (k + 1) * CH, :].bitcast(bf16),
                            idxs_ap=et[:],
                            num_idxs=SE,
                            num_idxs_reg=int(cnt),
                            elem_size=2 * H,
                        )
                    if "convert" not in skip:
                        mb = msgbp.tile([128, TPS, H], bf16, tag="msgb")
                        nc.vector.tensor_copy(mb[:], mt[:])
                        msgb_tiles[k][sl] = mb
                    else:
                        msgb_tiles[k][sl] = None
                    # S tiles for this slice, built in SB-tile groups
                    tbase = int(TOFF[k]) + sl * TPS
                    for g in range(TPS // SB):
                        if "sbuild" in skip:
                            s_tiles[k][sl * (TPS // SB) + g] = None
                            continue
                        stile = sp.tile([128, SB, 128], bf16, tag="stile")
                        r = relv_t[
                            :, tbase + g * SB : tbase + (g + 1) * SB
                        ].unsqueeze(2).broadcast_to((128, SB, 128))
                        io = iota_t[:].unsqueeze(1).broadcast_to((128, SB, 128))
                        nc.vector.tensor_tensor(
                            stile[:], io, r, mybir.AluOpType.is_equal
                        )
                        s_tiles[k][sl * (TPS // SB) + g] = stile

                for b in range(NBLK):
                    ntile = int(sum(T_bk[k][b] for k in range(NCHUNK)))
                    if ntile == 0:
                        nc.vector.memset(aggT[:, b * 128 : (b + 1) * 128], 0.0)
                        continue
                    ps = aggps.tile([64, 128], f32, tag="aggps")
                    it = 0
                    for k in range(NCHUNK):
                        t0 = int(off_bk[k][b]) // 128
                        for i in range(int(T_bk[k][b])):
                            tk = t0 + i
                            sl, col = tk // TPS, tk % TPS
                            ensure_slice(k, sl)
                            mb = msgb_tiles[k][sl]
                            stile = s_tiles[k][tk // SB]
                            if "smm" not in skip and mb is not None and stile is not None:
                                nc.tensor.matmul(
                                    ps[0:H, :],
                                    mb[:, col, :],
                                    stile[:, tk % SB, :],
                                    start=(it == 0),
                                    stop=(it == ntile - 1),
                                )
                            it += 1
                    nc.vector.tensor_copy(
                        aggT[:, b * 128 : (b + 1) * 128], ps[0:H, :]
                    )
                # 4) dense update (in place on hT)
                for i in range(NDCH):
                    sl_ = np.s_[:, i * 512 : (i + 1) * 512]
                    ps = dps.tile([64, 512], f32, tag="dps")
                    nc.tensor.matmul(
                        ps[0:H, :], relWT_t[l][:], aggT[sl_], start=True, stop=False
                    )
                    nc.tensor.matmul(
                        ps[0:H, :], rootWT_t[l][:], hT[sl_], start=False, stop=True
                    )
                    nc.scalar.activation(
                        hT[sl_],
                        ps[0:H, :],
                        mybir.ActivationFunctionType.Relu,
                        bias=relb_t[l][:],
                    )

            # ---- post-MP dense ----
            for i in range(NDCH):
                ps = dps.tile([64, 512], f32, tag="dps")
                nc.tensor.matmul(
                    ps[0:out_dim, :],
                    postWT_t[:],
                    hT[:, i * 512 : (i + 1) * 512],
                    start=True,
                    stop=True,
                )
                ot = iop.tile([out_dim, 512], f32, tag="ot")
                nc.scalar.activation(
                    ot[:],
                    ps[0:out_dim, :],
                    mybir.ActivationFunctionType.Relu,
                    bias=postb_t[:],
                )
                nc.sync.dma_start(
                    out=outT_d[:, i * 512 : (i + 1) * 512], in_=ot[:]
                )

    nc.compile()
    return nc


# ----------------------------------------------------------------------------
# Entry point
# ----------------------------------------------------------------------------

def _make_in_maps(inputs, plan, ep, n_cores):
    import ml_dtypes

    x = np.asarray(inputs["x"], dtype=np.float32)
    n_nodes, in_dim = x.shape
    hidden = inputs["preW"].shape[0]
    out_dim = inputs["postW"].shape[0]
    n_mp = sum(1 for k in inputs if k.startswith("relW"))
    NL, NPAD = plan["NL"], plan["NPAD"]

    xpad = np.zeros((NPAD, in_dim), dtype=np.float32)
    xpad[:n_nodes] = x
    iota = (
        np.broadcast_to(np.arange(128, dtype=np.float32)[None, :], (128, 128))
        .copy()
        .astype(ml_dtypes.bfloat16)
    )
    ident = np.eye(hidden, dtype=np.float32)

    shared = {
        "iota": iota,
        "ident": ident,
        "preWT": np.ascontiguousarray(np.asarray(inputs["preW"], np.float32).T),
        "preb": np.asarray(inputs["preb"], np.float32).reshape(hidden, 1),
        "postWT": np.ascontiguousarray(np.asarray(inputs["postW"], np.float32).T),
        "postb": np.asarray(inputs["postb"], np.float32).reshape(out_dim, 1),
    }
    for l in range(n_mp):
        shared[f"relWT{l}"] = np.ascontiguousarray(
            np.asarray(inputs[f"relW{l}"], np.float32).T
        )
        shared[f"rootWT{l}"] = np.ascontiguousarray(
            np.asarray(inputs[f"rootW{l}"], np.float32).T
        )
        shared[f"relb{l}"] = np.asarray(inputs[f"relb{l}"], np.float32).reshape(
            hidden, 1
        )

    in_maps = []
    for c in range(n_cores):
        m = dict(shared)
        m["xT"] = np.ascontiguousarray(xpad[c * NL : (c + 1) * NL].T)
        m["relv"] = ep["relv"][c]
        for k in range(plan["NCHUNK"]):
            m[f"eidx{k}"] = ep["eidx"][c][k]
        in_maps.append(m)
    return in_maps


def _run(inputs, n_cores=8, trace=False):
    from concourse.bass_utils import run_bass_kernel_spmd

    x = np.asarray(inputs["x"], dtype=np.float32)
    edge_index = np.asarray(inputs["edge_index"])
    n_nodes, in_dim = x.shape
    n_edges = edge_index.shape[1]
    hidden = inputs["preW"].shape[0]
    out_dim = inputs["postW"].shape[0]
    n_mp = sum(1 for k in inputs if k.startswith("relW"))

    plan = _plan(n_nodes, n_edges, n_cores)
    ep = _preprocess_edges(edge_index, n_cores, plan)

    nc = _build_program(plan, ep, n_cores, n_mp, in_dim, out_dim, hidden)
    in_maps = _make_in_maps(inputs, plan, ep, n_cores)

    res = run_bass_kernel_spmd(
        nc, in_maps, list(range(n_cores)), trace=trace
    )
    outs = [res.results[c]["outT"] for c in range(n_cores)]
    full = np.concatenate([o.T for o in outs], axis=0)  # [NPAD, out_dim]
    return full[:n_nodes], res


def kernel(**inputs):
    out, _ = _run(inputs, n_cores=8)
    return out



# revision 5
# speedup vs baseline: 1.7686x; 1.7686x over previous
"""Trainium2 Bass kernel for a 3-layer GraphConv GNN (nn_CustomGNN_34050500722941).

Reference computation (per layer, PyG GraphConv aggr='add'):
    h = relu(x @ preW.T + preb)
    3x: h = relu(segment_sum(h[src], dst) @ relW.T + relb + h @ rootW.T)
    out = relu(h @ postW.T + postb)

Strategy (8 NeuronCores, SPMD):
  - Nodes are sharded by range: core c owns nodes [c*NL, (c+1)*NL).
  - The full node-feature table h is replicated in each core's DRAM via
    AllGather once per layer (node-major rows of 64 fp32 = 256B).
  - Each core processes the edges whose dst lands in its range:
      * messages h[src] are fetched with GPSIMD dma_gather (one 256B row
        per edge).  Gather indices are int16, so the table is split into
        NCHUNK row-ranges (<=32767 rows each) and edges are grouped by the
        chunk their src falls into.
      * the scatter-add over dst is performed on the PE: edges are sorted
        by 128-wide dst block; for each 128-edge tile a one-hot matrix
        S[e, dst_rel] (built on DVE with is_equal against an iota) maps
        messages into a PSUM accumulator [64 feat, 128 dst] (feature-major).
      * dense part: h_new.T = relu(relW.T.T @ agg.T + rootW.T.T @ h.T + b)
        entirely feature-major on the PE, no transposes needed.
  - Host does index preprocessing (edge bucketing/padding) and the final
    unshard/transpose.
"""

import math
import numpy as np


# ----------------------------------------------------------------------------
# Host-side preprocessing
# ----------------------------------------------------------------------------

def _plan(n_nodes, n_edges, n_cores):
    NL = ((n_nodes + n_cores - 1) // n_cores + 511) // 512 * 512
    NPAD = NL * n_cores
    NCHUNK = 1
    while NPAD // NCHUNK > 32512 or NPAD % NCHUNK:
        NCHUNK += 1
    CH = NPAD // NCHUNK
    NBLK = NL // 128
    NDCH = NL // 512
    return dict(NL=NL, NPAD=NPAD, NCHUNK=NCHUNK, CH=CH, NBLK=NBLK, NDCH=NDCH)


def _preprocess_edges(edge_index, n_cores, plan):
    """Bucket edges by (core, chunk, block); build gather index / rel-dst
    arrays with a layout shared by all cores (cell sizes = max over cores)."""
    NL, NCHUNK, CH, NBLK = plan["NL"], plan["NCHUNK"], plan["CH"], plan["NBLK"]
    SE = 1024  # edges per gather slice (SWDGE ring holds 1024 descriptors)

    src = np.asarray(edge_index[0], dtype=np.int64)
    dst = np.asarray(edge_index[1], dtype=np.int64)

    core = dst // NL
    chunk = src // CH
    src_local = (src % CH).astype(np.int32)
    dst_local = dst % NL
    block = (dst_local // 128).astype(np.int32)
    rel = (dst_local % 128).astype(np.int32)

    # counts[core, chunk, block]
    key = (core * NCHUNK + chunk) * NBLK + block
    counts = np.bincount(key, minlength=n_cores * NCHUNK * NBLK).reshape(
        n_cores, NCHUNK, NBLK
    )
    # tiles per cell, shared across cores
    T_bk = (counts.max(axis=0) + 127) // 128  # [NCHUNK, NBLK]
    cell_slots = T_bk * 128
    # slot offset of cell (k, b) within chunk-stream k
    off_bk = np.zeros((NCHUNK, NBLK), dtype=np.int64)
    off_bk[:, 1:] = np.cumsum(cell_slots, axis=1)[:, :-1]
    L_used = cell_slots.sum(axis=1)  # [NCHUNK] used slots per stream
    L_k = ((L_used + SE - 1) // SE * SE).astype(np.int64)  # padded to slices
    L_k = np.maximum(L_k, SE)

    # rank of each edge within its (core, chunk, block) cell
    order = np.lexsort((block, chunk, core))
    key_sorted = key[order]
    grp_start = np.zeros(len(key_sorted), dtype=np.int64)
    new_grp = np.empty(len(key_sorted), dtype=bool)
    if len(key_sorted):
        new_grp[0] = True
        new_grp[1:] = key_sorted[1:] != key_sorted[:-1]
        starts = np.flatnonzero(new_grp)
        grp_start = starts[np.cumsum(new_grp) - 1]
    rank_sorted = np.arange(len(key_sorted)) - grp_start

    rank = np.empty(len(src), dtype=np.int64)
    rank[order] = rank_sorted

    slot = off_bk[chunk, block] + rank  # slot within chunk-stream

    # Per-core arrays
    eidx = []  # [n_cores][NCHUNK] int16 [128, L_k/16]
    relv = []  # [n_cores] bf16-able fp32 [128, Ttot]
    TOFF = np.zeros(NCHUNK + 1, dtype=np.int64)
    for k in range(NCHUNK):
        TOFF[k + 1] = TOFF[k] + L_k[k] // 128
    Ttot = int(TOFF[NCHUNK])

    for c in range(n_cores):
        m_c = core == c
        e_c = []
        rv = np.full(Ttot * 128, -1.0, dtype=np.float32)
        for k in range(NCHUNK):
            m = m_c & (chunk == k)
            arr = np.zeros(L_k[k], dtype=np.int16)  # cell pads -> row 0
            arr[int(L_used[k]):] = -1  # stream tail -> skipped
            arr[slot[m]] = src_local[m].astype(np.int16)
            wrapped = arr.reshape(-1, 16).T  # [16, L/16]
            e_c.append(np.tile(wrapped, (8, 1)).copy())  # [128, L/16]
            rv[TOFF[k] * 128 + slot[m]] = rel[m]
        eidx.append(e_c)
        # slot i -> (partition i%128, tile i//128)
        import ml_dtypes

        relv.append(
            np.ascontiguousarray(rv.reshape(Ttot, 128).T).astype(ml_dtypes.bfloat16)
        )

    return dict(
        SE=SE,
        eidx=eidx,
        relv=relv,
        T_bk=T_bk.astype(int),
        off_bk=off_bk,
        L_used=L_used.astype(int),
        L_k=L_k.astype(int),
        TOFF=TOFF,
        Ttot=Ttot,
    )


# ----------------------------------------------------------------------------
# Device program
# ----------------------------------------------------------------------------

def _build_program(plan, ep, n_cores, n_mp, in_dim, out_dim, hidden, skip=(), repeats=1):
    import concourse.bass as bass
    import concourse.bacc as bacc
    import concourse.mybir as mybir
    from concourse import tile

    f32 = mybir.dt.float32
    bf16 = mybir.dt.bfloat16
    i16 = mybir.dt.int16
    H = hidden
    NL, NPAD, NCHUNK, CH = plan["NL"], plan["NPAD"], plan["NCHUNK"], plan["CH"]
    NBLK, NDCH = plan["NBLK"], plan["NDCH"]
    SE = ep["SE"]
    T_bk, off_bk, L_k, L_used = ep["T_bk"], ep["off_bk"], ep["L_k"], ep["L_used"]
    TOFF, Ttot = ep["TOFF"], ep["Ttot"]
    SB = 8  # tiles per S-build op (one gather slice)
    TPS = SE // 128  # tiles per slice (20)

    NQ = 4  # SWDGE queues (ucode max); gathers round-robin across them
    nc = bacc.Bacc(
        None, target_bir_lowering=False, num_devices=n_cores, num_swdge_queues=NQ
    )
    rg = [list(range(n_cores))]

    # ---- I/O ----
    xT_d = nc.dram_tensor("xT", [in_dim, NL], f32, kind="ExternalInput")
    eidx_d = [
        nc.dram_tensor(f"eidx{k}", [128, int(L_k[k]) // 16], i16, kind="ExternalInput")
        for k in range(NCHUNK)
    ]
    relv_d = nc.dram_tensor("relv", [128, Ttot], bf16, kind="ExternalInput")
    iota_d = nc.dram_tensor("iota", [128, 128], bf16, kind="ExternalInput")
    ident_d = nc.dram_tensor("ident", [H, H], f32, kind="ExternalInput")
    preWT_d = nc.dram_tensor("preWT", [in_dim, H], f32, kind="ExternalInput")
    preb_d = nc.dram_tensor("preb", [H, 1], f32, kind="ExternalInput")
    relWT_d = [
        nc.dram_tensor(f"relWT{l}", [H, H], f32, kind="ExternalInput")
        for l in range(n_mp)
    ]
    rootWT_d = [
        nc.dram_tensor(f"rootWT{l}", [H, H], f32, kind="ExternalInput")
        for l in range(n_mp)
    ]
    relb_d = [
        nc.dram_tensor(f"relb{l}", [H, 1], f32, kind="ExternalInput")
        for l in range(n_mp)
    ]
    postWT_d = nc.dram_tensor("postWT", [H, out_dim], f32, kind="ExternalInput")
    postb_d = nc.dram_tensor("postb", [out_dim, 1], f32, kind="ExternalInput")
    outT_d = nc.dram_tensor("outT", [out_dim, NL], f32, kind="ExternalOutput")

    # ---- internal DRAM (h tables per layer) ----
    tbl_loc = [nc.dram_tensor(f"tbl_loc{l}", [NL, H], f32) for l in range(n_mp)]
    tbl = [
        nc.dram_tensor(f"tbl{l}", [NPAD, H], f32, addr_space="Shared")
        for l in range(n_mp)
    ]

    with tile.TileContext(nc) as tc:
        with (
            tc.tile_pool(name="const", bufs=1) as constp,
            tc.tile_pool(name="big", bufs=1) as bigp,
            tc.tile_pool(name="msg", bufs=8) as msgp,
            tc.tile_pool(name="msgb", bufs=10) as msgbp,
            tc.tile_pool(name="sbuild", bufs=6) as sp,
            tc.tile_pool(name="eix", bufs=8) as eixp,
            tc.tile_pool(name="small", bufs=3) as smallp,
            tc.tile_pool(name="io", bufs=2) as iop,
            tc.tile_pool(name="aggps", bufs=4, space="PSUM") as aggps,
            tc.tile_pool(name="dps", bufs=2, space="PSUM") as dps,
            tc.tile_pool(name="tps", bufs=2, space="PSUM") as tps,
        ):
            # ---- resident constants ----
            iota_t = constp.tile([128, 128], bf16, tag="iota")
            nc.sync.dma_start(out=iota_t[:], in_=iota_d[:])
            ident_t = constp.tile([H, H], f32, tag="ident")
            nc.sync.dma_start(out=ident_t[:], in_=ident_d[:])
            preWT_t = constp.tile([in_dim, H], f32, tag="preWT")
            nc.sync.dma_start(out=preWT_t[:], in_=preWT_d[:])
            preb_t = constp.tile([H, 1], f32, tag="preb")
            nc.sync.dma_start(out=preb_t[:], in_=preb_d[:])
            postWT_t = constp.tile([H, out_dim], f32, tag="postWT")
            nc.sync.dma_start(out=postWT_t[:], in_=postWT_d[:])
            postb_t = constp.tile([out_dim, 1], f32, tag="postb")
            nc.sync.dma_start(out=postb_t[:], in_=postb_d[:])
            relWT_t, rootWT_t, relb_t = [], [], []
            for l in range(n_mp):
                w1 = constp.tile([H, H], f32, tag=f"relWT{l}")
                nc.sync.dma_start(out=w1[:], in_=relWT_d[l][:])
                w2 = constp.tile([H, H], f32, tag=f"rootWT{l}")
                nc.sync.dma_start(out=w2[:], in_=rootWT_d[l][:])
                b1 = constp.tile([H, 1], f32, tag=f"relb{l}")
                nc.sync.dma_start(out=b1[:], in_=relb_d[l][:])
                relWT_t.append(w1)
                rootWT_t.append(w2)
                relb_t.append(b1)
            relv_t = constp.tile([128, Ttot], bf16, tag="relv")
            nc.sync.dma_start(out=relv_t[:], in_=relv_d[:])

            hT_t = bigp.tile([H, NL], f32, tag="hT")
            aggT_t = bigp.tile([H, NL], f32, tag="aggT")
            hT = hT_t[:, :]
            aggT = aggT_t[:, :]

            # ---- pre-MP dense ----
            for i in range(NDCH):
                xt = iop.tile([in_dim, 512], f32, tag="xt")
                nc.sync.dma_start(out=xt[:], in_=xT_d[:, i * 512 : (i + 1) * 512])
                ps = dps.tile([64, 512], f32, tag="dps")
                nc.tensor.matmul(ps[0:H, :], preWT_t[:], xt[:], start=True, stop=True)
                nc.scalar.activation(
                    hT[:, i * 512 : (i + 1) * 512],
                    ps[0:H, :],
                    mybir.ActivationFunctionType.Relu,
                    bias=preb_t[:],
                )

            # ---- message-passing layers ----
            for l in [l_ for _ in range(repeats) for l_ in range(n_mp)]:
                # 1) node-major h table rows: transpose hT block-wise, DMA out
                tblr = tbl_loc[l].rearrange("(j p) f -> p j f", p=128)  # [128,NBLK,H]
                for j0 in range(0, NBLK, 4):
                    jn = min(4, NBLK - j0)
                    st = smallp.tile([128, 4, H], f32, tag="tblw")
                    for j in range(j0, j0 + jn):
                        pt = tps.tile([128, H], f32, tag="tps")
                        nc.tensor.transpose(
                            pt[:], hT[:, j * 128 : (j + 1) * 128], ident_t[:]
                        )
                        nc.scalar.activation(
                            st[:, j - j0, :],
                            pt[:],
                            mybir.ActivationFunctionType.Copy,
                        )
                    nc.sync.dma_start(
                        out=tblr[:, j0 : j0 + jn, :], in_=st[:, 0:jn, :]
                    )
                # 2) replicate across cores
                if "ag" not in skip:
                    nc.gpsimd.collective_compute(
                    "AllGather",
                    mybir.AluOpType.bypass,
                        replica_groups=rg,
                        ins=[tbl_loc[l][:]],
                        outs=[tbl[l][:]],
                    )
                # 3) gather + S-build + segment-matmul, block-major schedule
                msgb_tiles = [dict() for _ in range(NCHUNK)]  # slice -> bf16 tile
                s_tiles = [dict() for _ in range(NCHUNK)]  # sb-idx -> S tile
                gq = [0]  # round-robin SWDGE queue counter

                def ensure_slice(k, sl):
                    if sl in msgb_tiles[k]:
                        return
                    n_sl = int(L_used[k] + SE - 1) // SE
                    if sl >= n_sl:
                        return
                    rem = int(L_used[k]) - sl * SE
                    cnt = min(SE, rem)
                    et = eixp.tile([128, SE // 16], i16, tag="eix")
                    nc.sync.dma_start(
                        out=et[:],
                        in_=eidx_d[k][:, sl * (SE // 16) : (sl + 1) * (SE // 16)],
                    )
                    mt = msgp.tile([128, TPS, H], f32, tag="msg")
                    if cnt < SE:
                        nc.vector.memset(mt[:], 0.0)
                    if "gather" in skip:
                        msgb_tiles[k][sl] = None
                    else:
                        nc.gpsimd.dma_gather(
                        out_ap=mt[:].bitcast(bf16),
                        in_ap=tbl[l][k * CH : (k + 1) * CH, :].bitcast(bf16),
                            idxs_ap=et[:],
                            num_idxs=SE,
                            num_idxs_reg=int(cnt),
                            elem_size=2 * H,
                            queue_num=gq[0] % NQ,
                        )
                        gq[0] += 1
                    if "convert" not in skip:
                        mb = msgbp.tile([128, TPS, H], bf16, tag="msgb")
                        nc.vector.tensor_copy(mb[:], mt[:])
                        msgb_tiles[k][sl] = mb
                    else:
                        msgb_tiles[k][sl] = None
                    # S tiles for this slice, built in SB-tile groups
                    tbase = int(TOFF[k]) + sl * TPS
                    for g in range(TPS // SB):
                        if "sbuild" in skip:
                            s_tiles[k][sl * (TPS // SB) + g] = None
                            continue
                        stile = sp.tile([128, SB, 128], bf16, tag="stile")
                        r = relv_t[
                            :, tbase + g * SB : tbase + (g + 1) * SB
                        ].unsqueeze(2).broadcast_to((128, SB, 128))
                        io = iota_t[:].unsqueeze(1).broadcast_to((128, SB, 128))
                        nc.vector.tensor_tensor(
                            stile[:], io, r, mybir.AluOpType.is_equal
                        )
                        s_tiles[k][sl * (TPS // SB) + g] = stile

                for b in range(NBLK):
                    ntile = int(sum(T_bk[k][b] for k in range(NCHUNK)))
                    if ntile == 0:
                        nc.vector.memset(aggT[:, b * 128 : (b + 1) * 128], 0.0)
                        continue
                    ps = aggps.tile([64, 128], f32, tag="aggps")
                    it = 0
                    for k in range(NCHUNK):
                        t0 = int(off_bk[k][b]) // 128
                        for i in range(int(T_bk[k][b])):
                            tk = t0 + i
                            sl, col = tk // TPS, tk % TPS
                            ensure_slice(k, sl)
                            mb = msgb_tiles[k][sl]
                            stile = s_tiles[k][tk // SB]
                            if "smm" not in skip and mb is not None and stile is not None:
                                nc.tensor.matmul(
                                    ps[0:H, :],
                                    mb[:, col, :],
                                    stile[:, tk % SB, :],
                                    start=(it == 0),
                                    stop=(it == ntile - 1),
                                )
                            it += 1
                    nc.vector.tensor_copy(
                        aggT[:, b * 128 : (b + 1) * 128], ps[0:H, :]
                    )
                # 4) dense update (in place on hT)
                for i in range(NDCH):
                    sl_ = np.s_[:, i * 512 : (i + 1) * 512]
                    ps = dps.tile([64, 512], f32, tag="dps")
                    nc.tensor.matmul(
                        ps[0:H, :], relWT_t[l][:], aggT[sl_], start=True, stop=False
                    )
                    nc.tensor.matmul(
                        ps[0:H, :], rootWT_t[l][:], hT[sl_], start=False, stop=True
                    )
                    nc.scalar.activation(
                        hT[sl_],
                        ps[0:H, :],
                        mybir.ActivationFunctionType.Relu,
                        bias=relb_t[l][:],
                    )

            # ---- post-MP dense ----
            for i in range(NDCH):
                ps = dps.tile([64, 512], f32, tag="dps")
                nc.tensor.matmul(
                    ps[0:out_dim, :],
                    postWT_t[:],
                    hT[:, i * 512 : (i + 1) * 512],
                    start=True,
                    stop=True,
                )
                ot = iop.tile([out_dim, 512], f32, tag="ot")
                nc.scalar.activation(
                    ot[:],
                    ps[0:out_dim, :],
                    mybir.ActivationFunctionType.Relu,
                    bias=postb_t[:],
                )
                nc.sync.dma_start(
                    out=outT_d[:, i * 512 : (i + 1) * 512], in_=ot[:]
                )

    nc.compile()
    return nc


# ----------------------------------------------------------------------------
# Entry point
# ----------------------------------------------------------------------------

def _make_in_maps(inputs, plan, ep, n_cores):
    import ml_dtypes

    x = np.asarray(inputs["x"], dtype=np.float32)
    n_nodes, in_dim = x.shape
    hidden = inputs["preW"].shape[0]
    out_dim = inputs["postW"].shape[0]
    n_mp = sum(1 for k in inputs if k.startswith("relW"))
    NL, NPAD = plan["NL"], plan["NPAD"]

    xpad = np.zeros((NPAD, in_dim), dtype=np.float32)
    xpad[:n_nodes] = x
    iota = (
        np.broadcast_to(np.arange(128, dtype=np.float32)[None, :], (128, 128))
        .copy()
        .astype(ml_dtypes.bfloat16)
    )
    ident = np.eye(hidden, dtype=np.float32)

    shared = {
        "iota": iota,
        "ident": ident,
        "preWT": np.ascontiguousarray(np.asarray(inputs["preW"], np.float32).T),
        "preb": np.asarray(inputs["preb"], np.float32).reshape(hidden, 1),
        "postWT": np.ascontiguousarray(np.asarray(inputs["postW"], np.float32).T),
        "postb": np.asarray(inputs["postb"], np.float32).reshape(out_dim, 1),
    }
    for l in range(n_mp):
        shared[f"relWT{l}"] = np.ascontiguousarray(
            np.asarray(inputs[f"relW{l}"], np.float32).T
        )
        shared[f"rootWT{l}"] = np.ascontiguousarray(
            np.asarray(inputs[f"rootW{l}"], np.float32).T
        )
        shared[f"relb{l}"] = np.asarray(inputs[f"relb{l}"], np.float32).reshape(
            hidden, 1
        )

    in_maps = []
    for c in range(n_cores):
        m = dict(shared)
        m["xT"] = np.ascontiguousarray(xpad[c * NL : (c + 1) * NL].T)
        m["relv"] = ep["relv"][c]
        for k in range(plan["NCHUNK"]):
            m[f"eidx{k}"] = ep["eidx"][c][k]
        in_maps.append(m)
    return in_maps


def _run(inputs, n_cores=8, trace=False):
    from concourse.bass_utils import run_bass_kernel_spmd

    x = np.asarray(inputs["x"], dtype=np.float32)
    edge_index = np.asarray(inputs["edge_index"])
    n_nodes, in_dim = x.shape
    n_edges = edge_index.shape[1]
    hidden = inputs["preW"].shape[0]
    out_dim = inputs["postW"].shape[0]
    n_mp = sum(1 for k in inputs if k.startswith("relW"))

    plan = _plan(n_nodes, n_edges, n_cores)
    ep = _preprocess_edges(edge_index, n_cores, plan)

    nc = _build_program(plan, ep, n_cores, n_mp, in_dim, out_dim, hidden)
    in_maps = _make_in_maps(inputs, plan, ep, n_cores)

    res = run_bass_kernel_spmd(
        nc, in_maps, list(range(n_cores)), trace=trace
    )
    outs = [res.results[c]["outT"] for c in range(n_cores)]
    full = np.concatenate([o.T for o in outs], axis=0)  # [NPAD, out_dim]
    return full[:n_nodes], res


def kernel(**inputs):
    out, _ = _run(inputs, n_cores=8)
    return out



# revision 8
# speedup vs baseline: 1.7864x; 1.0100x over previous
"""Trainium2 Bass kernel for a 3-layer GraphConv GNN (nn_CustomGNN_34050500722941).

Reference computation (per layer, PyG GraphConv aggr='add'):
    h = relu(x @ preW.T + preb)
    3x: h = relu(segment_sum(h[src], dst) @ relW.T + relb + h @ rootW.T)
    out = relu(h @ postW.T + postb)

Strategy (8 NeuronCores, SPMD), v2:
  - Node g's owner: c = (g % 25600)//3200; its local row l = (g//25600)*3200
    + g%3200.  Each AllGather chunk k assembles table rows for global nodes
    [k*25600, (k+1)*25600) from every core's strip k (local rows
    [k*3200, (k+1)*3200)), so table AllGathers pipeline against gathers.
  - The per-layer node table is bf16 with 256B rows (64 feats + 64 pad):
    SWDGE dma_gather requires 256B-multiple elements; bf16 rows make the
    gathered messages matmul-ready with no f32->bf16 CAST pass.
  - Gathers round-robin over 4 SWDGE queues (desc-gen ~2.2ns/edge vs 8.5
    single-queue).  Edge slots are packed densely: cells keyed by
    (src chunk k, dst block b) at 64-slot granularity, sized by the max
    count over cores (shared SPMD layout).
  - Aggregation: per 128-slot tile, S[e, dst_rel] one-hot (DVE tensor_scalar
    is_equal vs iota) is the matmul lhsT, messages the rhs; PSUM accumulates
    node-major agg [128 dst, 64 f] per cell, then adds into agg_nm in SBUF
    (chunk streams are independent, so stream k runs right after AG k).
  - agg_nm blocks are PE-transposed to feature-major aggT for the dense
    update h = relu(relW@agg + rootW@h + b) in f32 (as [64, 512] chunks).
  - Tables are double-buffered across layers so layer l+1's AllGathers
    overlap layer l's tail.
"""

import numpy as np


N_CORES = 8
NL = 12800          # nodes per core
NPAD = 102400
KCH = 4             # gather/AG chunks
CHN = NPAD // KCH   # nodes per chunk (25600)
STRIP = NL // KCH   # local rows per strip (3200)
NBLK = NL // 128    # dst blocks per core (100)
BPS = STRIP // 128  # blocks per strip (25)
GRAN = 64           # cell slot granularity
SE = 1024           # slots per gather call (SWDGE ring depth)
TPS = SE // 128     # tiles per slice
H = 64
NDCH = NL // 512    # dense chunks (25)


def _node_map(g):
    """global node id -> (core, local row)"""
    k = g // CHN
    w = g % CHN
    return w // STRIP, k * STRIP + w % STRIP


def _preprocess_edges(edge_index, n_cores):
    import ml_dtypes

    src = np.asarray(edge_index[0], dtype=np.int64)
    dst = np.asarray(edge_index[1], dtype=np.int64)

    core, l = _node_map(dst)
    b = l // 128
    rel = l % 128
    k = src // CHN
    idx = (src % CHN).astype(np.int16)

    # counts[core, k, b]
    key = (core * KCH + k) * NBLK + b
    counts = np.bincount(key, minlength=n_cores * KCH * NBLK).reshape(
        n_cores, KCH, NBLK
    )
    cell_slots = (counts.max(axis=0) + GRAN - 1) // GRAN * GRAN  # [KCH, NBLK]
    off = np.zeros((KCH, NBLK), dtype=np.int64)
    off[:, 1:] = np.cumsum(cell_slots, axis=1)[:, :-1]
    L_used = cell_slots.sum(axis=1)
    L_k = np.maximum((L_used + SE - 1) // SE * SE, SE).astype(np.int64)
    TOFF = np.zeros(KCH + 1, dtype=np.int64)
    for kk in range(KCH):
        TOFF[kk + 1] = TOFF[kk] + L_k[kk] // 128
    Ttot = int(TOFF[KCH])

    # per-edge rank within its (core, k, b) cell
    order = np.lexsort((b, k, core))
    key_sorted = key[order]
    n = len(src)
    new_grp = np.empty(n, dtype=bool)
    new_grp[0] = True
    new_grp[1:] = key_sorted[1:] != key_sorted[:-1]
    starts = np.flatnonzero(new_grp)
    grp_start = starts[np.cumsum(new_grp) - 1]
    rank_sorted = np.arange(n) - grp_start
    rank = np.empty(n, dtype=np.int64)
    rank[order] = rank_sorted
    slot = off[k, b] + rank  # slot within chunk-stream k

    eidx, relv = [], []
    for c in range(n_cores):
        m_c = core == c
        e_c = []
        rv = np.full(Ttot * 128, -1.0, dtype=np.float32)
        for kk in range(KCH):
            m = m_c & (k == kk)
            arr = np.zeros(L_k[kk], dtype=np.int16)
            arr[int(L_used[kk]):] = -1  # stream tail -> trimmed
            arr[slot[m]] = idx[m]
            wrapped = arr.reshape(-1, 16).T  # [16, L/16]
            e_c.append(np.tile(wrapped, (8, 1)).copy())  # [128, L/16]
            rv[TOFF[kk] * 128 + slot[m]] = rel[m]
        eidx.append(e_c)
        relv.append(np.ascontiguousarray(rv.reshape(Ttot, 128).T))

    # shared subtile schedule: per (k, b) -> list of (tile_in_chunk, p0, kk_rows)
    sched = [[[] for _ in range(NBLK)] for _ in range(KCH)]
    for kk in range(KCH):
        for bb in range(NBLK):
            s = int(off[kk][bb])
            nslots = int(cell_slots[kk][bb])
            while nslots > 0:
                p0 = s % 128
                take = min(128 - p0, nslots)
                sched[kk][bb].append((s // 128, p0, take))
                s += take
                nslots -= take

    return dict(
        eidx=eidx, relv=relv, sched=sched,
        L_used=L_used.astype(int), L_k=L_k.astype(int), TOFF=TOFF, Ttot=Ttot,
    )


def _build_program(ep, n_cores, n_mp, in_dim, out_dim):
    import concourse.bass as bass
    import concourse.bacc as bacc
    import concourse.mybir as mybir
    from concourse import tile

    f32 = mybir.dt.float32
    bf16 = mybir.dt.bfloat16
    i16 = mybir.dt.int16
    L_k, L_used, TOFF, Ttot = ep["L_k"], ep["L_used"], ep["TOFF"], ep["Ttot"]
    sched = ep["sched"]
    NQ = 4

    nc = bacc.Bacc(
        None, target_bir_lowering=False, num_devices=n_cores, num_swdge_queues=NQ
    )
    rg = [list(range(n_cores))]

    # ---- I/O ----
    xT_d = nc.dram_tensor("xT", [in_dim, NL], f32, kind="ExternalInput")
    eidx_d = [
        nc.dram_tensor(f"eidx{k}", [128, int(L_k[k]) // 16], i16, kind="ExternalInput")
        for k in range(KCH)
    ]
    relv_d = nc.dram_tensor("relv", [128, Ttot], f32, kind="ExternalInput")
    iota_d = nc.dram_tensor("iota", [128, 128], bf16, kind="ExternalInput")
    id64_d = nc.dram_tensor("id64", [H, H], f32, kind="ExternalInput")
    id128_d = nc.dram_tensor("id128", [128, 128], f32, kind="ExternalInput")
    preWT_d = nc.dram_tensor("preWT", [in_dim, H], f32, kind="ExternalInput")
    preb_d = nc.dram_tensor("preb", [H, 1], f32, kind="ExternalInput")
    relWT_d = [
        nc.dram_tensor(f"relWT{l}", [H, H], f32, kind="ExternalInput")
        for l in range(n_mp)
    ]
    rootWT_d = [
        nc.dram_tensor(f"rootWT{l}", [H, H], f32, kind="ExternalInput")
        for l in range(n_mp)
    ]
    relb_d = [
        nc.dram_tensor(f"relb{l}", [H, 1], f32, kind="ExternalInput")
        for l in range(n_mp)
    ]
    postWT_d = nc.dram_tensor("postWT", [H, out_dim], f32, kind="ExternalInput")
    postb_d = nc.dram_tensor("postb", [out_dim, 1], f32, kind="ExternalInput")
    outT_d = nc.dram_tensor("outT", [out_dim, NL], f32, kind="ExternalOutput")

    # ---- internal DRAM: per-chunk tables, double-buffered across layers ----
    tbl_loc = [
        [nc.dram_tensor(f"tbl_loc{k}_{p}", [STRIP, 128], bf16) for k in range(KCH)]
        for p in range(2)
    ]
    tbl = [
        [
            nc.dram_tensor(f"tbl{k}_{p}", [CHN, 128], bf16, addr_space="Shared")
            for k in range(KCH)
        ]
        for p in range(2)
    ]

    with tile.TileContext(nc) as tc:
        with (
            tc.tile_pool(name="const", bufs=1) as constp,
            tc.tile_pool(name="big", bufs=1) as bigp,
            tc.tile_pool(name="msg", bufs=8) as msgp,
            tc.tile_pool(name="sbuild", bufs=8) as sp,
            tc.tile_pool(name="eix", bufs=8) as eixp,
            tc.tile_pool(name="strip", bufs=2) as stp,
            tc.tile_pool(name="io", bufs=2) as iop,
            tc.tile_pool(name="aggps", bufs=3, space="PSUM") as aggps,
            tc.tile_pool(name="dps", bufs=2, space="PSUM") as dps,
            tc.tile_pool(name="tps", bufs=2, space="PSUM") as tps,
            tc.tile_pool(name="tps2", bufs=1, space="PSUM") as tps2,
        ):
            # ---- resident constants ----
            iota_t = constp.tile([128, 128], bf16, tag="iota")
            nc.sync.dma_start(out=iota_t[:], in_=iota_d[:])
            id64_t = constp.tile([H, H], f32, tag="id64")
            nc.sync.dma_start(out=id64_t[:], in_=id64_d[:])
            id128_t = constp.tile([128, 128], f32, tag="id128")
            nc.sync.dma_start(out=id128_t[:], in_=id128_d[:])
            preWT_t = constp.tile([in_dim, H], f32, tag="preWT")
            nc.sync.dma_start(out=preWT_t[:], in_=preWT_d[:])
            preb_t = constp.tile([H, 1], f32, tag="preb")
            nc.sync.dma_start(out=preb_t[:], in_=preb_d[:])
            postWT_t = constp.tile([H, out_dim], f32, tag="postWT")
            nc.sync.dma_start(out=postWT_t[:], in_=postWT_d[:])
            postb_t = constp.tile([out_dim, 1], f32, tag="postb")
            nc.sync.dma_start(out=postb_t[:], in_=postb_d[:])
            relWT_t, rootWT_t, relb_t = [], [], []
            for l in range(n_mp):
                w1 = constp.tile([H, H], f32, tag=f"relWT{l}")
                nc.sync.dma_start(out=w1[:], in_=relWT_d[l][:])
                w2 = constp.tile([H, H], f32, tag=f"rootWT{l}")
                nc.sync.dma_start(out=w2[:], in_=rootWT_d[l][:])
                b1 = constp.tile([H, 1], f32, tag=f"relb{l}")
                nc.sync.dma_start(out=b1[:], in_=relb_d[l][:])
                relWT_t.append(w1)
                rootWT_t.append(w2)
                relb_t.append(b1)
            relv_t = constp.tile([128, Ttot], f32, tag="relv")
            nc.sync.dma_start(out=relv_t[:], in_=relv_d[:])

            hT_t = bigp.tile([H, NL], f32, tag="hT")
            aggT_t = bigp.tile([H, NL], f32, tag="aggT")
            agg_nm_t = bigp.tile([128, NBLK, H], f32, tag="agg_nm")
            hT = hT_t[:, :]
            aggT = aggT_t[:, :]

            # ---- pre-MP dense (feature-major) ----
            for i in range(NDCH):
                xt = iop.tile([in_dim, 512], f32, tag="xt")
                nc.sync.dma_start(out=xt[:], in_=xT_d[:, i * 512 : (i + 1) * 512])
                ps = dps.tile([64, 512], f32, tag="dps")
                nc.tensor.matmul(ps[0:H, :], preWT_t[:], xt[:], start=True, stop=True)
                nc.scalar.activation(
                    hT[:, i * 512 : (i + 1) * 512],
                    ps[0:H, :],
                    mybir.ActivationFunctionType.Relu,
                    bias=preb_t[:],
                )

            def write_strips(par):
                # hT -> bf16 node-major strips -> DRAM -> AllGather, per chunk
                for ks in range(KCH):
                    st = stp.tile([128, BPS, 128], bf16, tag="st")
                    nc.vector.memset(st[:], 0.0)
                    for j in range(BPS):
                        jj = ks * BPS + j
                        pt = tps.tile([128, H], f32, tag="tps")
                        nc.tensor.transpose(
                            pt[:], hT[:, jj * 128 : (jj + 1) * 128], id64_t[:]
                        )
                        nc.scalar.activation(
                            st[:, j, 0:H], pt[:], mybir.ActivationFunctionType.Copy
                        )
                    tblr = tbl_loc[par][ks].rearrange("(j p) f -> p j f", p=128)
                    nc.sync.dma_start(out=tblr[:, :, :], in_=st[:])
                    nc.gpsimd.collective_compute(
                        "AllGather",
                        mybir.AluOpType.bypass,
                        replica_groups=rg,
                        ins=[tbl_loc[par][ks][:]],
                        outs=[tbl[par][ks][:]],
                    )

            write_strips(0)

            gq = [0]  # SWDGE queue round-robin counter

            # ---- message-passing layers ----
            for l in range(n_mp):
                par = l % 2
                msg_tiles = [dict() for _ in range(KCH)]  # slice -> msg tile
                s_tiles = [dict() for _ in range(KCH)]  # tile_in_chunk -> S tile

                def ensure_slice(k, sl, msg_tiles=msg_tiles, par=par):
                    if sl in msg_tiles[k]:
                        return
                    rem = int(L_used[k]) - sl * SE
                    cnt = min(SE, rem)
                    et = eixp.tile([128, SE // 16], i16, tag="eix")
                    nc.sync.dma_start(
                        out=et[:],
                        in_=eidx_d[k][:, sl * (SE // 16) : (sl + 1) * (SE // 16)],
                    )
                    mt = msgp.tile([128, TPS, 128], bf16, tag="msg")
                    if cnt < SE:
                        nc.vector.memset(mt[:], 0.0)
                    nc.gpsimd.dma_gather(
                        out_ap=mt[:],
                        in_ap=tbl[par][k][:],
                        idxs_ap=et[:],
                        num_idxs=SE,
                        num_idxs_reg=int(cnt),
                        elem_size=128,
                        queue_num=gq[0] % NQ,
                    )
                    gq[0] += 1
                    msg_tiles[k][sl] = mt

                def ensure_stile(k, tk, s_tiles=s_tiles):
                    if tk in s_tiles[k]:
                        return
                    stile = sp.tile([128, 128], bf16, tag="stile")
                    t_abs = int(TOFF[k]) + tk
                    nc.vector.tensor_scalar(
                        stile[:],
                        iota_t[:],
                        relv_t[:, t_abs : t_abs + 1],
                        None,
                        mybir.AluOpType.is_equal,
                    )
                    s_tiles[k][tk] = stile

                # chunk-major streams so stream k starts right after AG k
                first_k = [None] * NBLK
                for k in range(KCH):
                    for bb in range(NBLK):
                        subs = sched[k][bb]
                        if not subs:
                            continue
                        ps = aggps.tile([128, H], f32, tag="aggps")
                        for i, (tk, p0, kk_rows) in enumerate(subs):
                            sl = tk // TPS
                            ensure_slice(k, sl)
                            ensure_stile(k, tk)
                            mt = msg_tiles[k][sl]
                            stile = s_tiles[k][tk]
                            col = tk % TPS
                            nc.tensor.matmul(
                                ps[:],
                                stile[p0 : p0 + kk_rows, :],
                                mt[p0 : p0 + kk_rows, col, 0:H],
                                start=(i == 0),
                                stop=(i == len(subs) - 1),
                            )
                        if first_k[bb] is None:
                            first_k[bb] = k
                            nc.vector.tensor_copy(agg_nm_t[:, bb, :], ps[:])
                        else:
                            nc.vector.tensor_tensor(
                                agg_nm_t[:, bb, :],
                                agg_nm_t[:, bb, :],
                                ps[:],
                                mybir.AluOpType.add,
                            )
                for bb in range(NBLK):
                    if first_k[bb] is None:
                        nc.vector.memset(agg_nm_t[:, bb, :], 0.0)

                # agg_nm -> aggT (feature-major) via PE transpose
                for bb in range(NBLK):
                    pt2 = tps2.tile([H, 128], f32, tag="tps2")
                    nc.tensor.transpose(pt2[:], agg_nm_t[:, bb, :], id128_t[:])
                    nc.scalar.activation(
                        aggT[:, bb * 128 : (bb + 1) * 128],
                        pt2[:],
                        mybir.ActivationFunctionType.Copy,
                    )

                # dense update (in place on hT)
                for i in range(NDCH):
                    sl_ = np.s_[:, i * 512 : (i + 1) * 512]
                    ps = dps.tile([64, 512], f32, tag="dps")
                    nc.tensor.matmul(
                        ps[0:H, :], relWT_t[l][:], aggT[sl_], start=True, stop=False
                    )
                    nc.tensor.matmul(
                        ps[0:H, :], rootWT_t[l][:], hT[sl_], start=False, stop=True
                    )
                    nc.scalar.activation(
                        hT[sl_],
                        ps[0:H, :],
                        mybir.ActivationFunctionType.Relu,
                        bias=relb_t[l][:],
                    )

                if l + 1 < n_mp:
                    write_strips((l + 1) % 2)

            # ---- post-MP dense ----
            for i in range(NDCH):
                ps = dps.tile([64, 512], f32, tag="dps")
                nc.tensor.matmul(
                    ps[0:out_dim, :],
                    postWT_t[:],
                    hT[:, i * 512 : (i + 1) * 512],
                    start=True,
                    stop=True,
                )
                ot = iop.tile([out_dim, 512], f32, tag="ot")
                nc.scalar.activation(
                    ot[:],
                    ps[0:out_dim, :],
                    mybir.ActivationFunctionType.Relu,
                    bias=postb_t[:],
                )
                nc.sync.dma_start(
                    out=outT_d[:, i * 512 : (i + 1) * 512], in_=ot[:]
                )

    nc.compile()
    return nc


# ----------------------------------------------------------------------------
# Entry point
# ----------------------------------------------------------------------------

def _perm_for_core(c):
    l = np.arange(NL)
    return (l // STRIP) * CHN + c * STRIP + (l % STRIP)


def _make_in_maps(inputs, ep, n_cores):
    import ml_dtypes

    x = np.asarray(inputs["x"], dtype=np.float32)
    n_nodes, in_dim = x.shape
    hidden = inputs["preW"].shape[0]
    out_dim = inputs["postW"].shape[0]
    n_mp = sum(1 for k in inputs if k.startswith("relW"))

    xpad = np.zeros((NPAD, in_dim), dtype=np.float32)
    xpad[:n_nodes] = x
    iota = (
        np.broadcast_to(np.arange(128, dtype=np.float32)[None, :], (128, 128))
        .copy()
        .astype(ml_dtypes.bfloat16)
    )

    shared = {
        "iota": iota,
        "id64": np.eye(hidden, dtype=np.float32),
        "id128": np.eye(128, dtype=np.float32),
        "preWT": np.ascontiguousarray(np.asarray(inputs["preW"], np.float32).T),
        "preb": np.asarray(inputs["preb"], np.float32).reshape(hidden, 1),
        "postWT": np.ascontiguousarray(np.asarray(inputs["postW"], np.float32).T),
        "postb": np.asarray(inputs["postb"], np.float32).reshape(out_dim, 1),
    }
    for l in range(n_mp):
        shared[f"relWT{l}"] = np.ascontiguousarray(
            np.asarray(inputs[f"relW{l}"], np.float32).T
        )
        shared[f"rootWT{l}"] = np.ascontiguousarray(
            np.asarray(inputs[f"rootW{l}"], np.float32).T
        )
        shared[f"relb{l}"] = np.asarray(inputs[f"relb{l}"], np.float32).reshape(
            hidden, 1
        )

    in_maps = []
    for c in range(n_cores):
        m = dict(shared)
        m["xT"] = np.ascontiguousarray(xpad[_perm_for_core(c)].T)
        m["relv"] = ep["relv"][c]
        for k in range(KCH):
            m[f"eidx{k}"] = ep["eidx"][c][k]
        in_maps.append(m)
    return in_maps


def _run(inputs, n_cores=8, trace=False):
    from concourse.bass_utils import run_bass_kernel_spmd

    x = np.asarray(inputs["x"], dtype=np.float32)
    edge_index = np.asarray(inputs["edge_index"])
    n_nodes, in_dim = x.shape
    out_dim = inputs["postW"].shape[0]
    n_mp = sum(1 for k in inputs if k.startswith("relW"))

    ep = _preprocess_edges(edge_index, n_cores)
    nc = _build_program(ep, n_cores, n_mp, in_dim, out_dim)
    in_maps = _make_in_maps(inputs, ep, n_cores)

    res = run_bass_kernel_spmd(nc, in_maps, list(range(n_cores)), trace=trace)
    full = np.zeros((NPAD, out_dim), dtype=np.float32)
    for c in range(n_cores):
        full[_perm_for_core(c)] = res.results[c]["outT"].T
    return full[:n_nodes], res


def kernel(**inputs):
    out, _ = _run(inputs, n_cores=8)
    return out


# revision 11
# speedup vs baseline: 2.2302x; 1.2485x over previous
"""Trainium2 Bass kernel for a 3-layer GraphConv GNN (nn_CustomGNN_34050500722941).

Reference computation (per layer, PyG GraphConv aggr='add'):
    h = relu(x @ preW.T + preb)
    3x: h = relu(segment_sum(h[src], dst) @ relW.T + relb + h @ rootW.T)
    out = relu(h @ postW.T + postb)

Strategy (8 NeuronCores, SPMD), v2:
  - Node g's owner: c = (g % 25600)//3200; its local row l = (g//25600)*3200
    + g%3200.  Each AllGather chunk k assembles table rows for global nodes
    [k*25600, (k+1)*25600) from every core's strip k (local rows
    [k*3200, (k+1)*3200)), so table AllGathers pipeline against gathers.
  - The per-layer node table is bf16 with 256B rows (64 feats + 64 pad):
    SWDGE dma_gather requires 256B-multiple elements; bf16 rows make the
    gathered messages matmul-ready with no f32->bf16 CAST pass.
  - Gathers round-robin over 4 SWDGE queues (desc-gen ~2.2ns/edge vs 8.5
    single-queue).  Edge slots are packed densely: cells keyed by
    (src chunk k, dst block b) at 64-slot granularity, sized by the max
    count over cores (shared SPMD layout).
  - Aggregation: per 128-slot tile, S[e, dst_rel] one-hot (DVE tensor_scalar
    is_equal vs iota) is the matmul lhsT, messages the rhs; PSUM accumulates
    node-major agg [128 dst, 64 f] per cell, then adds into agg_nm in SBUF
    (chunk streams are independent, so stream k runs right after AG k).
  - agg_nm blocks are PE-transposed to feature-major aggT for the dense
    update h = relu(relW@agg + rootW@h + b) in f32 (as [64, 512] chunks).
  - Tables are double-buffered across layers so layer l+1's AllGathers
    overlap layer l's tail.
"""

import numpy as np


N_CORES = 8
NL = 12800          # nodes per core
NPAD = 102400
KCH = 4             # gather/AG chunks
CHN = NPAD // KCH   # nodes per chunk (25600)
STRIP = NL // KCH   # local rows per strip (3200)
NBLK = NL // 128    # dst blocks per core (100)
BPS = STRIP // 128  # blocks per strip (25)
GRAN = 64           # cell slot granularity
SE = 1024           # slots per gather call (SWDGE ring depth)
TPS = SE // 128     # tiles per slice
H = 64
NDCH = NL // 512    # dense chunks (25)


def _node_map(g):
    """global node id -> (core, local row)"""
    k = g // CHN
    w = g % CHN
    return w // STRIP, k * STRIP + w % STRIP


def _preprocess_edges(edge_index, n_cores):
    import ml_dtypes

    src = np.asarray(edge_index[0], dtype=np.int64)
    dst = np.asarray(edge_index[1], dtype=np.int64)

    core, l = _node_map(dst)
    b = l // 128
    rel = l % 128
    k = src // CHN
    idx = (src % CHN).astype(np.int16)

    # counts[core, k, b]
    key = (core * KCH + k) * NBLK + b
    counts = np.bincount(key, minlength=n_cores * KCH * NBLK).reshape(
        n_cores, KCH, NBLK
    )
    cell_slots = (counts.max(axis=0) + GRAN - 1) // GRAN * GRAN  # [KCH, NBLK]
    off = np.zeros((KCH, NBLK), dtype=np.int64)
    off[:, 1:] = np.cumsum(cell_slots, axis=1)[:, :-1]
    L_used = cell_slots.sum(axis=1)
    L_k = np.maximum((L_used + SE - 1) // SE * SE, SE).astype(np.int64)
    TOFF = np.zeros(KCH + 1, dtype=np.int64)
    for kk in range(KCH):
        TOFF[kk + 1] = TOFF[kk] + L_k[kk] // 128
    Ttot = int(TOFF[KCH])

    # per-edge rank within its (core, k, b) cell
    order = np.lexsort((b, k, core))
    key_sorted = key[order]
    n = len(src)
    new_grp = np.empty(n, dtype=bool)
    new_grp[0] = True
    new_grp[1:] = key_sorted[1:] != key_sorted[:-1]
    starts = np.flatnonzero(new_grp)
    grp_start = starts[np.cumsum(new_grp) - 1]
    rank_sorted = np.arange(n) - grp_start
    rank = np.empty(n, dtype=np.int64)
    rank[order] = rank_sorted
    slot = off[k, b] + rank  # slot within chunk-stream k

    eidx, relv = [], []
    for c in range(n_cores):
        m_c = core == c
        e_c = []
        rv = np.full(Ttot * 128, -1.0, dtype=np.float32)
        for kk in range(KCH):
            m = m_c & (k == kk)
            arr = np.zeros(L_k[kk], dtype=np.int16)
            arr[int(L_used[kk]):] = -1  # stream tail -> trimmed
            arr[slot[m]] = idx[m]
            wrapped = arr.reshape(-1, 16).T  # [16, L/16]
            e_c.append(np.tile(wrapped, (8, 1)).copy())  # [128, L/16]
            rv[TOFF[kk] * 128 + slot[m]] = rel[m]
        eidx.append(e_c)
        relv.append(
            np.ascontiguousarray(rv.reshape(Ttot, 128).T).astype(ml_dtypes.bfloat16)
        )

    # shared subtile schedule: per (k, b) -> list of (tile_in_chunk, p0, kk_rows)
    sched = [[[] for _ in range(NBLK)] for _ in range(KCH)]
    for kk in range(KCH):
        for bb in range(NBLK):
            s = int(off[kk][bb])
            nslots = int(cell_slots[kk][bb])
            while nslots > 0:
                p0 = s % 128
                take = min(128 - p0, nslots)
                sched[kk][bb].append((s // 128, p0, take))
                s += take
                nslots -= take

    return dict(
        eidx=eidx, relv=relv, sched=sched,
        L_used=L_used.astype(int), L_k=L_k.astype(int), TOFF=TOFF, Ttot=Ttot,
    )


def _build_program(ep, n_cores, n_mp, in_dim, out_dim):
    import concourse.bass as bass
    import concourse.bacc as bacc
    import concourse.mybir as mybir
    from concourse import tile

    f32 = mybir.dt.float32
    bf16 = mybir.dt.bfloat16
    i16 = mybir.dt.int16
    L_k, L_used, TOFF, Ttot = ep["L_k"], ep["L_used"], ep["TOFF"], ep["Ttot"]
    sched = ep["sched"]
    NQ = 4

    nc = bacc.Bacc(
        None, target_bir_lowering=False, num_devices=n_cores, num_swdge_queues=NQ
    )
    rg = [list(range(n_cores))]

    # ---- I/O ----
    xT_d = nc.dram_tensor("xT", [in_dim, NL], f32, kind="ExternalInput")
    eidx_d = [
        nc.dram_tensor(f"eidx{k}", [128, int(L_k[k]) // 16], i16, kind="ExternalInput")
        for k in range(KCH)
    ]
    relv_d = nc.dram_tensor("relv", [128, Ttot], bf16, kind="ExternalInput")
    iota_d = nc.dram_tensor("iota", [128, 128], bf16, kind="ExternalInput")
    id64_d = nc.dram_tensor("id64", [H, H], f32, kind="ExternalInput")
    id128_d = nc.dram_tensor("id128", [128, 128], f32, kind="ExternalInput")
    preWT_d = nc.dram_tensor("preWT", [in_dim, H], f32, kind="ExternalInput")
    preb_d = nc.dram_tensor("preb", [H, 1], f32, kind="ExternalInput")
    relWT_d = [
        nc.dram_tensor(f"relWT{l}", [H, H], f32, kind="ExternalInput")
        for l in range(n_mp)
    ]
    rootWT_d = [
        nc.dram_tensor(f"rootWT{l}", [H, H], f32, kind="ExternalInput")
        for l in range(n_mp)
    ]
    relb_d = [
        nc.dram_tensor(f"relb{l}", [H, 1], f32, kind="ExternalInput")
        for l in range(n_mp)
    ]
    postWT_d = nc.dram_tensor("postWT", [H, out_dim], f32, kind="ExternalInput")
    postb_d = nc.dram_tensor("postb", [out_dim, 1], f32, kind="ExternalInput")
    outT_d = nc.dram_tensor("outT", [out_dim, NL], f32, kind="ExternalOutput")

    # ---- internal DRAM: per-chunk tables, double-buffered across layers ----
    tbl_loc = [
        [nc.dram_tensor(f"tbl_loc{k}_{p}", [STRIP, 128], bf16) for k in range(KCH)]
        for p in range(2)
    ]
    tbl = [
        [
            nc.dram_tensor(f"tbl{k}_{p}", [CHN, 128], bf16, addr_space="Shared")
            for k in range(KCH)
        ]
        for p in range(2)
    ]

    with tile.TileContext(nc) as tc:
        with (
            tc.tile_pool(name="const", bufs=1) as constp,
            tc.tile_pool(name="big", bufs=1) as bigp,
            tc.tile_pool(name="msg", bufs=8) as msgp,
            tc.tile_pool(name="sbuild", bufs=8) as sp,
            tc.tile_pool(name="eix", bufs=8) as eixp,
            tc.tile_pool(name="strip", bufs=2) as stp,
            tc.tile_pool(name="io", bufs=2) as iop,
            tc.tile_pool(name="aggps", bufs=3, space="PSUM") as aggps,
            tc.tile_pool(name="dps", bufs=2, space="PSUM") as dps,
            tc.tile_pool(name="tps", bufs=1, space="PSUM") as tps,
            tc.tile_pool(name="tps2", bufs=2, space="PSUM") as tps2,
        ):
            # ---- resident constants ----
            iota_t = constp.tile([128, 128], bf16, tag="iota")
            nc.sync.dma_start(out=iota_t[:], in_=iota_d[:])
            id64_t = constp.tile([H, H], f32, tag="id64")
            nc.sync.dma_start(out=id64_t[:], in_=id64_d[:])
            id128_t = constp.tile([128, 128], f32, tag="id128")
            nc.sync.dma_start(out=id128_t[:], in_=id128_d[:])
            preWT_t = constp.tile([in_dim, H], f32, tag="preWT")
            nc.sync.dma_start(out=preWT_t[:], in_=preWT_d[:])
            preb_t = constp.tile([H, 1], f32, tag="preb")
            nc.sync.dma_start(out=preb_t[:], in_=preb_d[:])
            postWT_t = constp.tile([H, out_dim], f32, tag="postWT")
            nc.sync.dma_start(out=postWT_t[:], in_=postWT_d[:])
            postb_t = constp.tile([out_dim, 1], f32, tag="postb")
            nc.sync.dma_start(out=postb_t[:], in_=postb_d[:])
            relWT_t, rootWT_t, relb_t = [], [], []
            for l in range(n_mp):
                w1 = constp.tile([H, H], f32, tag=f"relWT{l}")
                nc.sync.dma_start(out=w1[:], in_=relWT_d[l][:])
                w2 = constp.tile([H, H], f32, tag=f"rootWT{l}")
                nc.sync.dma_start(out=w2[:], in_=rootWT_d[l][:])
                b1 = constp.tile([H, 1], f32, tag=f"relb{l}")
                nc.sync.dma_start(out=b1[:], in_=relb_d[l][:])
                relWT_t.append(w1)
                rootWT_t.append(w2)
                relb_t.append(b1)
            relv_t = constp.tile([128, Ttot], bf16, tag="relv")
            nc.sync.dma_start(out=relv_t[:], in_=relv_d[:])

            hT_t = bigp.tile([H, NL], f32, tag="hT")
            aggT_t = bigp.tile([H, NL], f32, tag="aggT")
            agg_nm_t = bigp.tile([128, NBLK, H], f32, tag="agg_nm")
            hT = hT_t[:, :]
            aggT = aggT_t[:, :]

            # ---- pre-MP dense (feature-major) ----
            for i in range(NDCH):
                xt = iop.tile([in_dim, 512], f32, tag="xt")
                nc.sync.dma_start(out=xt[:], in_=xT_d[:, i * 512 : (i + 1) * 512])
                ps = dps.tile([64, 512], f32, tag="dps")
                nc.tensor.matmul(ps[0:H, :], preWT_t[:], xt[:], start=True, stop=True)
                nc.scalar.activation(
                    hT[:, i * 512 : (i + 1) * 512],
                    ps[0:H, :],
                    mybir.ActivationFunctionType.Relu,
                    bias=preb_t[:],
                )

            def emit_strip(par, ks):
                # hT strip -> bf16 node-major rows -> DRAM -> AllGather
                st = stp.tile([128, BPS, 128], bf16, tag="st")
                nc.vector.memset(st[:], 0.0)
                for j in range(BPS):
                    jj = ks * BPS + j
                    pt = tps.tile([128, H], f32, tag="tps")
                    nc.tensor.transpose(
                        pt[:], hT[:, jj * 128 : (jj + 1) * 128], id64_t[:]
                    )
                    nc.scalar.activation(
                        st[:, j, 0:H], pt[:], mybir.ActivationFunctionType.Copy
                    )
                tblr = tbl_loc[par][ks].rearrange("(j p) f -> p j f", p=128)
                nc.sync.dma_start(out=tblr[:, :, :], in_=st[:])
                nc.gpsimd.collective_compute(
                    "AllGather",
                    mybir.AluOpType.bypass,
                    replica_groups=rg,
                    ins=[tbl_loc[par][ks][:]],
                    outs=[tbl[par][ks][:]],
                )

            for ks in range(KCH):
                emit_strip(0, ks)

            gq = [0]  # SWDGE queue round-robin counter
            # last chunk stream contributing to each block (None = no edges)
            last_k = [None] * NBLK
            for bb in range(NBLK):
                for k in range(KCH):
                    if sched[k][bb]:
                        last_k[bb] = k
            # dense chunk i needs blocks 4i..4i+3; strip ks needs dense
            # chunks up to ceil((ks+1)*STRIP/512)-1
            strip_after_dense = [
                ((ks + 1) * STRIP + 511) // 512 - 1 for ks in range(KCH)
            ]

            def emit_transpose(bb):
                pt2 = tps2.tile([H, 128], f32, tag="tps2")
                nc.tensor.transpose(pt2[:], agg_nm_t[:, bb, :], id128_t[:])
                nc.scalar.activation(
                    aggT[:, bb * 128 : (bb + 1) * 128],
                    pt2[:],
                    mybir.ActivationFunctionType.Copy,
                )

            def emit_dense(l, i):
                sl_ = np.s_[:, i * 512 : (i + 1) * 512]
                ps = dps.tile([64, 512], f32, tag="dps")
                nc.tensor.matmul(
                    ps[0:H, :], relWT_t[l][:], aggT[sl_], start=True, stop=False
                )
                nc.tensor.matmul(
                    ps[0:H, :], rootWT_t[l][:], hT[sl_], start=False, stop=True
                )
                nc.scalar.activation(
                    hT[sl_],
                    ps[0:H, :],
                    mybir.ActivationFunctionType.Relu,
                    bias=relb_t[l][:],
                )

            def emit_post(i):
                ps = dps.tile([64, 512], f32, tag="dps")
                nc.tensor.matmul(
                    ps[0:out_dim, :],
                    postWT_t[:],
                    hT[:, i * 512 : (i + 1) * 512],
                    start=True,
                    stop=True,
                )
                ot = iop.tile([out_dim, 512], f32, tag="ot")
                nc.scalar.activation(
                    ot[:],
                    ps[0:out_dim, :],
                    mybir.ActivationFunctionType.Relu,
                    bias=postb_t[:],
                )
                nc.sync.dma_start(
                    out=outT_d[:, i * 512 : (i + 1) * 512], in_=ot[:]
                )

            # ---- message-passing layers ----
            for l in range(n_mp):
                par = l % 2
                msg_tiles = [dict() for _ in range(KCH)]  # slice -> msg tile
                s_tiles = [dict() for _ in range(KCH)]  # slice -> S tiles [128,TPS,128]

                def ensure_slice(k, sl, msg_tiles=msg_tiles, s_tiles=s_tiles, par=par):
                    if sl in msg_tiles[k]:
                        return
                    rem = int(L_used[k]) - sl * SE
                    cnt = min(SE, rem)
                    et = eixp.tile([128, SE // 16], i16, tag="eix")
                    nc.sync.dma_start(
                        out=et[:],
                        in_=eidx_d[k][:, sl * (SE // 16) : (sl + 1) * (SE // 16)],
                    )
                    mt = msgp.tile([128, TPS, 128], bf16, tag="msg")
                    if cnt < SE:
                        nc.vector.memset(mt[:], 0.0)
                    nc.gpsimd.dma_gather(
                        out_ap=mt[:],
                        in_ap=tbl[par][k][:],
                        idxs_ap=et[:],
                        num_idxs=SE,
                        num_idxs_reg=int(cnt),
                        elem_size=128,
                        queue_num=gq[0] % NQ,
                    )
                    gq[0] += 1
                    msg_tiles[k][sl] = mt
                    # one batched is_equal builds all TPS S tiles of the slice
                    stile = sp.tile([128, TPS, 128], bf16, tag="stile")
                    tbase = int(TOFF[k]) + sl * TPS
                    r = relv_t[:, tbase : tbase + TPS].unsqueeze(2).broadcast_to(
                        (128, TPS, 128)
                    )
                    io = iota_t[:].unsqueeze(1).broadcast_to((128, TPS, 128))
                    nc.vector.tensor_tensor(
                        stile[:], io, r, mybir.AluOpType.is_equal
                    )
                    s_tiles[k][sl] = stile

                # blocks with no edges at all: zero agg + transpose up front
                for bb in range(NBLK):
                    if last_k[bb] is None:
                        nc.vector.memset(agg_nm_t[:, bb, :], 0.0)
                        emit_transpose(bb)

                # interleaved completion tracking for the dense pipeline
                blocks_left = [
                    sum(1 for bb in range(i * 4, i * 4 + 4) if last_k[bb] is not None)
                    for i in range(NDCH)
                ]
                dense_left = [strip_after_dense[ks] + 1 for ks in range(KCH)]
                first_k = [None] * NBLK

                def block_done(bb, l=l, par=par, dense_left=dense_left,
                               blocks_left=blocks_left):
                    emit_transpose(bb)
                    i = bb // 4
                    blocks_left[i] -= 1
                    if blocks_left[i] == 0:
                        emit_dense(l, i)
                        if l + 1 == n_mp:
                            emit_post(i)
                        else:
                            for ks in range(KCH):
                                if strip_after_dense[ks] >= i:
                                    dense_left[ks] -= 1
                                    if dense_left[ks] == 0:
                                        emit_strip((l + 1) % 2, ks)

                # chunk-major streams so stream k starts right after AG k
                for k in range(KCH):
                    for bb in range(NBLK):
                        subs = sched[k][bb]
                        if not subs:
                            continue
                        ps = aggps.tile([128, H], f32, tag="aggps")
                        for i, (tk, p0, kk_rows) in enumerate(subs):
                            sl = tk // TPS
                            ensure_slice(k, sl)
                            mt = msg_tiles[k][sl]
                            stile = s_tiles[k][sl]
                            col = tk % TPS
                            nc.tensor.matmul(
                                ps[:],
                                stile[p0 : p0 + kk_rows, col, :],
                                mt[p0 : p0 + kk_rows, col, 0:H],
                                start=(i == 0),
                                stop=(i == len(subs) - 1),
                            )
                        if first_k[bb] is None:
                            first_k[bb] = k
                            nc.scalar.activation(
                                agg_nm_t[:, bb, :],
                                ps[:],
                                mybir.ActivationFunctionType.Copy,
                            )
                        else:
                            nc.vector.tensor_tensor(
                                agg_nm_t[:, bb, :],
                                agg_nm_t[:, bb, :],
                                ps[:],
                                mybir.AluOpType.add,
                            )
                        if k == last_k[bb]:
                            block_done(bb)

    nc.compile()
    return nc


# ----------------------------------------------------------------------------
# Entry point
# ----------------------------------------------------------------------------

def _perm_for_core(c):
    l = np.arange(NL)
    return (l // STRIP) * CHN + c * STRIP + (l % STRIP)


def _make_in_maps(inputs, ep, n_cores):
    import ml_dtypes

    x = np.asarray(inputs["x"], dtype=np.float32)
    n_nodes, in_dim = x.shape
    hidden = inputs["preW"].shape[0]
    out_dim = inputs["postW"].shape[0]
    n_mp = sum(1 for k in inputs if k.startswith("relW"))

    xpad = np.zeros((NPAD, in_dim), dtype=np.float32)
    xpad[:n_nodes] = x
    iota = (
        np.broadcast_to(np.arange(128, dtype=np.float32)[None, :], (128, 128))
        .copy()
        .astype(ml_dtypes.bfloat16)
    )

    shared = {
        "iota": iota,
        "id64": np.eye(hidden, dtype=np.float32),
        "id128": np.eye(128, dtype=np.float32),
        "preWT": np.ascontiguousarray(np.asarray(inputs["preW"], np.float32).T),
        "preb": np.asarray(inputs["preb"], np.float32).reshape(hidden, 1),
        "postWT": np.ascontiguousarray(np.asarray(inputs["postW"], np.float32).T),
        "postb": np.asarray(inputs["postb"], np.float32).reshape(out_dim, 1),
    }
    for l in range(n_mp):
        shared[f"relWT{l}"] = np.ascontiguousarray(
            np.asarray(inputs[f"relW{l}"], np.float32).T
        )
        shared[f"rootWT{l}"] = np.ascontiguousarray(
            np.asarray(inputs[f"rootW{l}"], np.float32).T
        )
        shared[f"relb{l}"] = np.asarray(inputs[f"relb{l}"], np.float32).reshape(
            hidden, 1
        )

    in_maps = []
    for c in range(n_cores):
        m = dict(shared)
        m["xT"] = np.ascontiguousarray(xpad[_perm_for_core(c)].T)
        m["relv"] = ep["relv"][c]
        for k in range(KCH):
            m[f"eidx{k}"] = ep["eidx"][c][k]
        in_maps.append(m)
    return in_maps


def _run(inputs, n_cores=8, trace=False):
    from concourse.bass_utils import run_bass_kernel_spmd

    x = np.asarray(inputs["x"], dtype=np.float32)
    edge_index = np.asarray(inputs["edge_index"])
    n_nodes, in_dim = x.shape
    out_dim = inputs["postW"].shape[0]
    n_mp = sum(1 for k in inputs if k.startswith("relW"))

    ep = _preprocess_edges(edge_index, n_cores)
    nc = _build_program(ep, n_cores, n_mp, in_dim, out_dim)
    in_maps = _make_in_maps(inputs, ep, n_cores)

    res = run_bass_kernel_spmd(nc, in_maps, list(range(n_cores)), trace=trace)
    full = np.zeros((NPAD, out_dim), dtype=np.float32)
    for c in range(n_cores):
        full[_perm_for_core(c)] = res.results[c]["outT"].T
    return full[:n_nodes], res


def kernel(**inputs):
    out, _ = _run(inputs, n_cores=8)
    return out
